# revision 1
# baseline (speedup 1.0000x reference)
"""Trainium2 Bass kernel for nn_Covariance_Metric (5-way 5-shot covariance metric).

Math (see reference):
  cov_w = centered-support covariance (512x512) per way w  [5 ways, 2205 samples each]
  sim[q,w,i] = q_i^T cov_w q_i   for each of 441 spatial positions i of query q
  scores[q,w] = conv_w . leaky_relu(sim[q,w,:]) + conv_b

Strategy: data-parallel over Q across 8 cores (19 queries/core, zero-padded).
Each core computes all 5 covariances (replicated) then its query shard.
Matmuls run in float32r (TF32-like, 1 cycle/row at N>=256).
sim uses DVE scalar_tensor_tensor fused multiply + free-dim reduce with the
query transposed on-chip via PE so that spatial position i is the partition dim.
"""
import json
import numpy as np
from contextlib import ExitStack

import concourse.bass as bass
import concourse.tile as tile
from concourse import mybir
from concourse.masks import make_identity

# ---------------------------------------------------------------------------
# Workaround for this container's walrus build: it supports only ONE sync-wait
# command per instruction, while Tile attaches several. Rewrite the BIR JSON to
# hoist extra waits onto inserted same-engine NoOps (the NX sequencer processes
# them in order, so the gating is equivalent).
# ---------------------------------------------------------------------------

def _split_sync_waits_json(m: dict) -> int:
    nsplit = 0
    for fn in m["functions"]:
        for bb in fn["blocks"]:
            out = []
            for ins in bb["instructions"]:
                si = ins.get("sync_info")
                if si:
                    w = si.get("on_wait") or []
                    if len(w) > 1:
                        for c in w[:-1]:
                            nsplit += 1
                            out.append({
                                "debug": ins.get("debug", 0),
                                "engine": ins["engine"],
                                "ins": [], "outs": [],
                                "name": f"{ins['name']}-ws{nsplit}",
                                "opcode": "NoOp",
                                "sync_info": {"on_wait": [c], "on_update": []},
                            })
                        si["on_wait"] = [w[-1]]
                out.append(ins)
            bb["instructions"] = out
    return nsplit


_fixups_installed = False


def _install_fixups():
    global _fixups_installed
    if _fixups_installed:
        return
    _fixups_installed = True
    import concourse.bass_utils as bu
    import concourse.bass2jax as b2j

    orig = bu.compile_bir_kernel

    def compile_bir_kernel_patched(bir_json, tmpdir, neff_name="file.neff"):
        m = json.loads(bir_json)
        _split_sync_waits_json(m)
        return orig(json.dumps(m).encode(), tmpdir, neff_name)

    bu.compile_bir_kernel = compile_bir_kernel_patched
    b2j.compile_bir_kernel = compile_bir_kernel_patched


# ---------------------------------------------------------------------------
# Problem constants (hardcoded per contract)
# ---------------------------------------------------------------------------
Q, C, HW = 150, 512, 441
WAY, SHOT = 5, 5
NCORES = 8
QP = 19           # queries per core (8*19 = 152 >= 150, zero-padded)
CC = C // 128     # 4 c-chunks
ISZ = [128, 128, 128, HW - 3 * 128]   # i/hw chunk sizes: 128,128,128,57
NS = SHOT * HW    # 2205 samples per way
F32 = mybir.dt.float32
F32R = mybir.dt.float32r

_cache = {}


def _build(nq=QP, do_cov=True):
    nc = bass.Bass(trn_type="TRN2")
    x1s = nc.dram_tensor("x1s", [QP, C, HW], F32, kind="ExternalInput")
    x2 = nc.dram_tensor("x2", [WAY * SHOT, C, HW], F32, kind="ExternalInput")
    cw = nc.dram_tensor("cw", [HW], F32, kind="ExternalInput")
    cb = nc.dram_tensor("cb", [1], F32, kind="ExternalInput")
    scores = nc.dram_tensor("scores", [QP, WAY], F32, kind="ExternalOutput")

    AL = mybir.AluOpType
    AF = mybir.ActivationFunctionType

    with tile.TileContext(nc) as tc, ExitStack() as ctx:
        consts = ctx.enter_context(tc.tile_pool(name="consts", bufs=1))
        tr_ps = ctx.enter_context(tc.tile_pool(name="tr_ps", bufs=2, space="PSUM"))

        ident = consts.tile([128, 128], F32)
        make_identity(nc, ident[:])
        cwt = consts.tile([128, 4], F32)
        nc.vector.memset(cwt[:], 0.0)
        for m in range(4):
            nc.sync.dma_start(cwt[: ISZ[m], m : m + 1], cw[m * 128 : m * 128 + ISZ[m]][:, None])
        cbt = consts.tile([1, 1], F32)
        nc.sync.dma_start(cbt[:], cb[None, :])
        cov = consts.tile([128, WAY, CC, C], F32R)   # 40KB/partition

        # ---------------- covariance phase ----------------
        with tc.tile_pool(name="x2w", bufs=2) as x2pool, \
             tc.tile_pool(name="fT", bufs=3) as fTp, \
             tc.tile_pool(name="musml", bufs=2) as mup, \
             tc.tile_pool(name="murow", bufs=2) as murp, \
             tc.tile_pool(name="g_ps", bufs=4, space="PSUM") as g_ps, \
             tc.tile_pool(name="mu_ps", bufs=1, space="PSUM") as mu_ps:
            for w in range(WAY if do_cov else 0):
                x2w = x2pool.tile([128, SHOT, CC, HW], F32)
                nc.sync.dma_start(
                    x2w[:], x2[w * SHOT : (w + 1) * SHOT].rearrange("s (cc p) hw -> p s cc hw", p=128)
                )
                # channel sums over (shot, hw): view free dims as (cc, s, hw), reduce XY
                musum = mup.tile([128, CC], F32)
                nc.vector.tensor_reduce(
                    out=musum[:], in_=x2w[:].rearrange("p s cc hw -> p cc s hw"),
                    axis=mybir.AxisListType.XY, op=AL.add,
                )
                g = [g_ps.tile([128, C], F32, name=f"g{w}_{j}", tag="g") for j in range(CC)]
                first = True
                for s in range(SHOT):
                    for h in range(4):
                        hsz = ISZ[h]
                        tp = tr_ps.tile([128, C], F32)
                        for j in range(CC):
                            nc.tensor.transpose(
                                tp[:hsz, j * 128 : (j + 1) * 128],
                                x2w[:, s, j, h * 128 : h * 128 + hsz], ident[:],
                            )
                        fT = fTp.tile([128, C], F32R)
                        nc.scalar.copy(fT[:hsz, :], tp[:hsz, :])
                        for j in range(CC):
                            nc.tensor.matmul(
                                g[j][:, :],
                                lhsT=fT[:hsz, j * 128 : (j + 1) * 128],
                                rhs=fT[:hsz, :],
                                start=first, stop=False,
                            )
                        first = False
                # mean row: transpose channel sums to a (1, 512) row
                mrow_ps = mu_ps.tile([1, C], F32)
                for j in range(CC):
                    nc.tensor.transpose(mrow_ps[0:1, j * 128 : (j + 1) * 128], musum[:, j : j + 1], ident[:])
                murow = murp.tile([1, C], F32R)   # mu = sums / NS
                nc.scalar.activation(murow[:], mrow_ps[:], AF.Copy, scale=1.0 / NS)
                mursc = murp.tile([1, C], F32R)   # -NS*mu = -sums
                nc.scalar.activation(mursc[:], mrow_ps[:], AF.Copy, scale=-1.0)
                for j in range(CC):
                    nc.tensor.matmul(
                        g[j][:, :],
                        lhsT=mursc[0:1, j * 128 : (j + 1) * 128],
                        rhs=murow[0:1, :],
                        start=False, stop=True,
                    )
                for j in range(CC):
                    nc.scalar.activation(cov[:, w, j, :], g[j][:, :], AF.Copy, scale=1.0 / (NS - 1))

        # ---------------- query phase ----------------
        with tc.tile_pool(name="qn", bufs=3) as qnp, \
             tc.tile_pool(name="qT", bufs=8) as qTp, \
             tc.tile_pool(name="prod", bufs=2) as prodp, \
             tc.tile_pool(name="sims", bufs=2) as simsp, \
             tc.tile_pool(name="orow", bufs=2) as orowp, \
             tc.tile_pool(name="qc_ps", bufs=4, space="PSUM") as qc_ps, \
             tc.tile_pool(name="sc_ps", bufs=2, space="PSUM") as sc_ps:
            for qi in range(nq):
                qn = qnp.tile([128, CC, HW], F32)
                nc.sync.dma_start(qn[:], x1s[qi].rearrange("(cc p) hw -> p cc hw", p=128))
                # center each channel over spatial positions: q -= mean_hw(q)
                qsum = simsp.tile([128, CC], F32)
                nc.vector.tensor_reduce(out=qsum[:], in_=qn[:], axis=mybir.AxisListType.X, op=AL.add)
                qmneg = simsp.tile([128, CC], F32)
                nc.scalar.activation(qmneg[:], qsum[:], AF.Copy, scale=-1.0 / HW)
                qcn = qnp.tile([128, CC, HW], F32R)
                for j in range(CC):
                    nc.scalar.activation(
                        qcn[:, j, :], qn[:, j, :], AF.Identity,
                        bias=qmneg[:, j : j + 1], scale=1.0,
                    )
                qn = qcn
                qT = []
                for m in range(4):
                    sz = ISZ[m]
                    tp = tr_ps.tile([128, C], F32)
                    for j in range(CC):
                        nc.tensor.transpose(
                            tp[:sz, j * 128 : (j + 1) * 128],
                            qn[:, j, m * 128 : m * 128 + sz].bitcast(F32),
                            ident[:],
                        )
                    t = qTp.tile([128, C], F32)
                    nc.scalar.copy(t[:sz, :], tp[:sz, :])
                    qT.append(t)
                S = simsp.tile([128, 20], F32)   # columns: m*5 + w
                nc.vector.memset(S[:], 0.0)
                for w in range(WAY):
                    for m in range(4):
                        sz = ISZ[m]
                        qc = qc_ps.tile([128, C], F32)
                        for j in range(CC):
                            nc.tensor.matmul(
                                qc[:sz, :],
                                lhsT=qn[:, j, m * 128 : m * 128 + sz],
                                rhs=cov[:, w, j, :],
                                start=(j == 0), stop=(j == CC - 1),
                            )
                        prod = prodp.tile([128, C], F32)
                        nc.vector.scalar_tensor_tensor(
                            out=prod[:sz, :], in0=qc[:sz, :], scalar=1.0, in1=qT[m][:sz, :],
                            op0=AL.mult, op1=AL.mult,
                            accum_out=S[:sz, m * 5 + w : m * 5 + w + 1],
                        )
                A = simsp.tile([128, 20], F32)
                nc.scalar.activation(A[:], S[:], AF.Lrelu, alpha=0.2)
                scp = sc_ps.tile([1, WAY], F32)
                for m in range(4):
                    nc.tensor.matmul(
                        scp[0:1, :], lhsT=cwt[:, m : m + 1], rhs=A[:, m * 5 : (m + 1) * 5],
                        start=(m == 0), stop=(m == 3),
                    )
                orow = orowp.tile([1, WAY], F32)
                nc.scalar.activation(orow[:], scp[:], AF.Identity, bias=cbt[0:1, 0:1], scale=1.0)
                nc.sync.dma_start(scores[qi : qi + 1, :], orow[0:1, :])
    return nc


def _get_nc(nq=QP, do_cov=True):
    key = ("nc", nq, do_cov)
    if key not in _cache:
        _install_fixups()
        _cache[key] = _build(nq, do_cov)
    return _cache[key]


def kernel(x1, x2, conv_w, conv_b, _trace=False):
    from concourse.bass_utils import run_bass_kernel_spmd

    nc = _get_nc()
    x1 = np.ascontiguousarray(np.asarray(x1, dtype=np.float32)).reshape(Q, C, HW)
    x2 = np.ascontiguousarray(np.asarray(x2, dtype=np.float32)).reshape(WAY * SHOT, C, HW)
    conv_w = np.asarray(conv_w, dtype=np.float32).reshape(HW)
    conv_b = np.asarray(conv_b, dtype=np.float32).reshape(1)

    x1p = np.zeros((NCORES * QP, C, HW), dtype=np.float32)
    x1p[:Q] = x1
    in_maps = []
    for c in range(NCORES):
        in_maps.append({
            "x1s": np.ascontiguousarray(x1p[c * QP : (c + 1) * QP]),
            "x2": x2, "cw": conv_w, "cb": conv_b,
        })
    res = run_bass_kernel_spmd(nc, in_maps, core_ids=list(range(NCORES)), trace=_trace)
    out = np.concatenate([res.results[c]["scores"] for c in range(NCORES)], axis=0)[:Q]
    if _trace:
        _cache["last_result"] = res
    return np.ascontiguousarray(out)



# revision 9
# speedup vs baseline: 1.4968x; 1.4968x over previous
"""Trainium2 Bass kernel for nn_Covariance_Metric (5-way 5-shot covariance metric).

Math (see reference):
  cov_w  = centered-support covariance (512x512) per way w  [2205 samples each]
  sim[q,w,i] = q_i^T cov_w q_i  (positions i of query q), then
  scores[q,w] = conv_w . leaky_relu(sim[q,w,:]) + conv_b

Key optimization: cov_w is PSD so sim >= 0 always (empirically sim in
[365, 689]); LeakyReLU is the identity.  Then
  scores[q,w] = sum_i w_i q_i^T cov_w q_i = <cov_w, Wq>_F,
  Wq = q diag(conv_w) q^T   (per query only -- 5x fewer MACs than per (q,w)).
Both cov_w and Wq are symmetric: only the 10 lower-triangular 128x128 blocks
are computed (packed 1280 cols), with strict-lower cov blocks pre-scaled 2x.
All matmuls/transposes run in bf16 (1 cycle/row at any output width).
Frobenius products run on DVE as bf16 SBUF ops (2x/4x DVE modes).

Data-parallel over Q across 8 cores (19 queries/core, zero-padded).
"""
import json
import numpy as np
from contextlib import ExitStack

import concourse.bass as bass
import concourse.tile as tile
from concourse import mybir
from concourse.masks import make_identity

# ---------------------------------------------------------------------------
# Workaround for this container's walrus build: it supports only ONE sync-wait
# command per instruction, while Tile attaches several. Rewrite the BIR JSON to
# hoist extra waits onto inserted same-engine NoOps (the NX sequencer processes
# them in order, so the gating is equivalent).
# ---------------------------------------------------------------------------

def _split_sync_waits_json(m: dict) -> int:
    nsplit = 0
    for fn in m["functions"]:
        for bb in fn["blocks"]:
            out = []
            for ins in bb["instructions"]:
                si = ins.get("sync_info")
                if si:
                    w = si.get("on_wait") or []
                    if len(w) > 1:
                        for c in w[:-1]:
                            nsplit += 1
                            out.append({
                                "debug": ins.get("debug", 0),
                                "engine": ins["engine"],
                                "ins": [], "outs": [],
                                "name": f"{ins['name']}-ws{nsplit}",
                                "opcode": "NoOp",
                                "sync_info": {"on_wait": [c], "on_update": []},
                            })
                        si["on_wait"] = [w[-1]]
                out.append(ins)
            bb["instructions"] = out
    return nsplit


_fixups_installed = False


def _install_fixups():
    global _fixups_installed
    if _fixups_installed:
        return
    _fixups_installed = True
    import concourse.bass_utils as bu
    import concourse.bass2jax as b2j

    orig = bu.compile_bir_kernel

    def compile_bir_kernel_patched(bir_json, tmpdir, neff_name="file.neff"):
        m = json.loads(bir_json)
        _split_sync_waits_json(m)
        return orig(json.dumps(m).encode(), tmpdir, neff_name)

    bu.compile_bir_kernel = compile_bir_kernel_patched
    b2j.compile_bir_kernel = compile_bir_kernel_patched


# ---------------------------------------------------------------------------
# Problem constants (hardcoded per contract)
# ---------------------------------------------------------------------------
Q, C, HW = 150, 512, 441
WAY, SHOT = 5, 5
NCORES = 8
QP = 19             # queries per core (8*19 = 152 >= 150, zero-padded)
CC = C // 128       # 4 channel chunks
HWP = 512           # hw padded to 4 chunks of 128
NS = SHOT * HW      # 2205 samples per way
TRI_OFF = [0, 128, 384, 768]   # packed col offset of block-row jc (width (jc+1)*128)
TRI_W = 1280
F32 = mybir.dt.float32
BF16 = mybir.dt.bfloat16

_cache = {}


def _build():
    nc = bass.Bass(trn_type="TRN2")
    x1s = nc.dram_tensor("x1s", [QP, C, HW], F32, kind="ExternalInput")
    x2 = nc.dram_tensor("x2", [WAY * SHOT, C, HW], F32, kind="ExternalInput")
    cw = nc.dram_tensor("cw", [HW], F32, kind="ExternalInput")
    cb = nc.dram_tensor("cb", [1], F32, kind="ExternalInput")
    scores = nc.dram_tensor("scores", [QP, WAY], F32, kind="ExternalOutput")

    AL = mybir.AluOpType
    AF = mybir.ActivationFunctionType

    with tile.TileContext(nc) as tc, ExitStack() as ctx:
        consts = ctx.enter_context(tc.tile_pool(name="consts", bufs=1))
        tr_ps = ctx.enter_context(tc.tile_pool(name="tr_ps", bufs=2, space="PSUM"))

        identB = consts.tile([128, 128], BF16)
        make_identity(nc, identB[:])
        onesB = consts.tile([128, 1], BF16)
        nc.vector.memset(onesB[:], 1.0)
        # conv_w as 4 per-partition columns (i-chunk m -> col m), zero-padded
        cwt = consts.tile([128, 4], F32)
        nc.vector.memset(cwt[:], 0.0)
        for m in range(4):
            sz = min(128, HW - m * 128)
            nc.sync.dma_start(cwt[:sz, m : m + 1], cw[m * 128 : m * 128 + sz][:, None])
        cbt = consts.tile([1, 1], F32)
        nc.sync.dma_start(cbt[:], cb[None, :])
        # packed lower-tri covariance, strict-lower pre-scaled 2x, all /(NS-1)
        covtri = consts.tile([128, WAY, TRI_W], BF16)

        # ---------------- covariance phase ----------------
        with tc.tile_pool(name="x2w", bufs=2) as x2wp, \
             tc.tile_pool(name="x2b", bufs=2) as x2bp, \
             tc.tile_pool(name="fT", bufs=3) as fTp, \
             tc.tile_pool(name="mus", bufs=2) as mup, \
             tc.tile_pool(name="g_ps", bufs=4, space="PSUM") as g_ps:
            for w in range(WAY):
                xw = x2wp.tile([128, SHOT, CC, HW], F32)
                nc.sync.dma_start(
                    xw[:], x2[w * SHOT : (w + 1) * SHOT].rearrange("s (cc p) hw -> p s cc hw", p=128)
                )
                # per-channel sums over (shot, hw); split cc chunks DVE/Pool
                csum = mup.tile([128, CC], F32)
                nc.vector.tensor_reduce(
                    out=csum[:], in_=xw[:].rearrange("p s cc hw -> p cc s hw"),
                    axis=mybir.AxisListType.XY, op=AL.add,
                )
                negmu = mup.tile([128, CC], F32)
                nc.scalar.activation(negmu[:], csum[:], AF.Copy, scale=-1.0 / NS)
                # centered bf16 copy, hw padded to 512 with zeros
                x2b = x2bp.tile([128, SHOT, CC, HWP], BF16)
                nc.vector.memset(x2b[:, :, :, HW:HWP], 0.0)
                for s in range(SHOT):
                    for cc in range(CC):
                        nc.scalar.activation(
                            x2b[:, s, cc, 0:HW], xw[:, s, cc, :], AF.Identity,
                            bias=negmu[:, cc : cc + 1], scale=1.0,
                        )
                g = [g_ps.tile([128, (jc + 1) * 128], F32, name=f"g{w}_{jc}", tag="g")
                     for jc in range(CC)]
                nchunk = 0
                for s in range(SHOT):
                    for m in range(4):
                        tp = tr_ps.tile([128, C], BF16)
                        for cc in range(CC):
                            nc.tensor.transpose(
                                tp[:, cc * 128 : (cc + 1) * 128],
                                x2b[:, s, cc, m * 128 : (m + 1) * 128], identB[:],
                            )
                        fT = fTp.tile([128, C], BF16)
                        if nchunk % 4 == 3:
                            nc.scalar.copy(out=fT[:], in_=tp[:])
                        else:
                            nc.vector.tensor_copy(out=fT[:], in_=tp[:])
                        for jc in range(CC):
                            nc.tensor.matmul(
                                g[jc][:, :],
                                lhsT=fT[:, jc * 128 : (jc + 1) * 128],
                                rhs=fT[:, 0 : (jc + 1) * 128],
                                start=(nchunk == 0), stop=(nchunk == SHOT * 4 - 1),
                            )
                        nchunk += 1
                # scale + pack into covtri (strict-lower x2, diag x1, /(NS-1))
                for jc in range(CC):
                    off = TRI_OFF[jc]
                    if jc > 0:
                        nc.scalar.activation(
                            covtri[:, w, off : off + jc * 128],
                            g[jc][:, 0 : jc * 128], AF.Copy, scale=2.0 / (NS - 1),
                        )
                    nc.scalar.activation(
                        covtri[:, w, off + jc * 128 : off + (jc + 1) * 128],
                        g[jc][:, jc * 128 : (jc + 1) * 128], AF.Copy,
                        scale=1.0 / (NS - 1),
                    )

        # ---------------- query phase ----------------
        with tc.tile_pool(name="qn", bufs=3) as qnp, \
             tc.tile_pool(name="qb", bufs=2) as qbp, \
             tc.tile_pool(name="qT", bufs=2) as qTp, \
             tc.tile_pool(name="wqs", bufs=2) as wqp, \
             tc.tile_pool(name="sS", bufs=2) as sSp, \
             tc.tile_pool(name="junk", bufs=1) as junkp, \
             tc.tile_pool(name="orow", bufs=2) as orowp, \
             tc.tile_pool(name="wq_ps", bufs=4, space="PSUM") as wq_ps, \
             tc.tile_pool(name="sc_ps", bufs=2, space="PSUM") as sc_ps:
            junk = junkp.tile([128, TRI_W], BF16)
            for qi in range(QP):
                qn = qnp.tile([128, CC, HW], F32)
                nc.sync.dma_start(qn[:], x1s[qi].rearrange("(cc p) hw -> p cc hw", p=128))
                qsum = sSp.tile([128, CC], F32)
                nc.vector.tensor_reduce(
                    out=qsum[:], in_=qn[:], axis=mybir.AxisListType.X, op=AL.add,
                )
                negmuq = sSp.tile([128, CC], F32)
                nc.scalar.activation(negmuq[:], qsum[:], AF.Copy, scale=-1.0 / HW)
                qnb = qbp.tile([128, CC, HWP], BF16)
                nc.vector.memset(qnb[:, :, HW:HWP], 0.0)
                for cc in range(CC):
                    nc.scalar.activation(
                        qnb[:, cc, 0:HW], qn[:, cc, :], AF.Identity,
                        bias=negmuq[:, cc : cc + 1], scale=1.0,
                    )
                # transpose to [i-part, c] and conv_w-scaled copy
                qT = qTp.tile([128, 4, C], BF16)
                qTw = qTp.tile([128, 4, C], BF16)
                for m in range(4):
                    tp = tr_ps.tile([128, C], BF16)
                    for cc in range(CC):
                        nc.tensor.transpose(
                            tp[:, cc * 128 : (cc + 1) * 128],
                            qnb[:, cc, m * 128 : (m + 1) * 128], identB[:],
                        )
                    nc.vector.tensor_copy(out=qT[:, m, :], in_=tp[:])
                    nc.vector.tensor_scalar(
                        out=qTw[:, m, :], in0=qT[:, m, :],
                        scalar1=cwt[:, m : m + 1], scalar2=None, op0=AL.mult,
                    )
                # Wq lower-tri blocks: out chunk cc covers d in [0, (cc+1)*128)
                wq = [wq_ps.tile([128, (cc + 1) * 128], F32, name=f"wq{qi}_{cc}", tag="wq")
                      for cc in range(CC)]
                for m in range(4):
                    for cc in range(CC):
                        nc.tensor.matmul(
                            wq[cc][:, :],
                            lhsT=qTw[:, m, cc * 128 : (cc + 1) * 128],
                            rhs=qT[:, m, 0 : (cc + 1) * 128],
                            start=(m == 0), stop=(m == 3),
                        )
                wqs = wqp.tile([128, TRI_W], BF16)
                for cc in range(CC):
                    nc.scalar.activation(
                        wqs[:, TRI_OFF[cc] : TRI_OFF[cc] + (cc + 1) * 128],
                        wq[cc][:, :], AF.Copy,
                    )
                # scores: Frobenius of packed tri blocks, per way
                S = sSp.tile([128, WAY], F32)
                for w in range(WAY):
                    nc.vector.scalar_tensor_tensor(
                        out=junk[:], in0=covtri[:, w, :], scalar=1.0, in1=wqs[:],
                        op0=AL.mult, op1=AL.mult,
                        accum_out=S[:, w : w + 1],
                    )
                srow = orowp.tile([1, WAY], F32)
                nc.gpsimd.tensor_reduce(
                    out=srow[0:1, :], in_=S[:], axis=mybir.AxisListType.C, op=AL.add,
                )
                orow = orowp.tile([1, WAY], F32)
                nc.scalar.activation(orow[:], srow[:], AF.Identity,
                                     bias=cbt[0:1, 0:1], scale=1.0)
                nc.sync.dma_start(scores[qi : qi + 1, :], orow[0:1, :])
    return nc


def _get_nc():
    if "nc" not in _cache:
        _install_fixups()
        _cache["nc"] = _build()
    return _cache["nc"]


def kernel(x1, x2, conv_w, conv_b, _trace=False):
    from concourse.bass_utils import run_bass_kernel_spmd

    nc = _get_nc()
    x1 = np.ascontiguousarray(np.asarray(x1, dtype=np.float32)).reshape(Q, C, HW)
    x2 = np.ascontiguousarray(np.asarray(x2, dtype=np.float32)).reshape(WAY * SHOT, C, HW)
    conv_w = np.asarray(conv_w, dtype=np.float32).reshape(HW)
    conv_b = np.asarray(conv_b, dtype=np.float32).reshape(1)

    x1p = np.zeros((NCORES * QP, C, HW), dtype=np.float32)
    x1p[:Q] = x1
    in_maps = []
    for c in range(NCORES):
        in_maps.append({
            "x1s": np.ascontiguousarray(x1p[c * QP : (c + 1) * QP]),
            "x2": x2, "cw": conv_w, "cb": conv_b,
        })
    res = run_bass_kernel_spmd(nc, in_maps, core_ids=list(range(NCORES)), trace=_trace)
    out = np.concatenate([res.results[c]["scores"] for c in range(NCORES)], axis=0)[:Q]
    if _trace:
        _cache["last_result"] = res
    return np.ascontiguousarray(out)
